# revision 18
# baseline (speedup 1.0000x reference)
"""Trainium2 Bass kernel for nn_Attention (dense transformer MHA block).

Reference computation (fp32):
    qkv = x @ w_qkv.T            # [B,N,3C]
    q,k,v per head; scores = q k^T / sqrt(D); attn = softmax(scores)
    o = attn @ v;  y = o @ w_proj.T + b_proj

Sharding over 8 NeuronCores (data-parallel over batch x tensor-parallel over
heads): core c -> (batch b = c//4, head group g = c%4, heads 4g..4g+3).
Each core computes q/k/v for its 4 heads over the full 2048-token sequence,
runs attention locally, and multiplies by its row-slice of w_proj, producing
a PARTIAL output [2048, 1024].  The 4 partials per batch are summed on the
host (numpy) together with the bias — no device collectives.

Perf structure (v2):
  - score matmuls run in 64x64 array-tiling mode: 4 concurrent tiles
    (2 heads x 2 kv-halves) per 512-wide round, ~2x faster than the
    untiled 64-contraction matmuls.
  - exp is split across engines: head A of each pair uses the ACT table
    exp; head B uses a Schraudolph bit-trick exp on the DVE
    (y_i16 = round(s*A + B) bitcast as bf16 == exp(s*scale) within ~4%).
    The per-head softmax normalization cancels each head's systematic
    scale error.
  - A@V keeps the ones-column trick (V gets a 65th column of ones so the
    same matmul accumulates the softmax denominator) - provably optimal
    since any separate denominator matmul would re-stream all exp
    columns through the PE.
  - x DMA is j(contraction-chunk)-major across two queues, with the
    first 512-token block of all chunks first so both the n-outer qk
    matmuls and the early v tiles never stall.
  - output projection for each 512-row block is emitted as soon as its
    two attention units finish (PE filler), normalize multiplies run on
    GPSIMD, PSUM->SBUF output copies on ACT.
"""

import numpy as np

B, N, C = 2, 2048, 1024
H, D = 16, 64
NCORES = 8
GROUPS = 4              # head groups (tensor-parallel)
HG = H // GROUPS        # 4 heads per core
CG = HG * D             # 256 channels per core
P = 128
KT = C // P             # 8 contraction subtiles for C=1024
KV_CHUNKS = N // P      # 16 key/value chunks of 128 rows
QT = N // 512           # 4 query tiles of 512
VB = D + 1              # v block width incl. ones column (65)
SCALE = 1.0 / float(np.sqrt(D))
# Schraudolph exp constants (bf16 bit-trick on DVE): for scores s (pre-scale),
# exp(s*SCALE) ~= bitcast_bf16(int16(s*EXP_A + EXP_B)).  The -7.63 centers the
# sawtooth approximation error; the per-head softmax cancels the global scale.
EXP_A = 128.0 * 1.4426950408889634 * SCALE
EXP_B = 127.0 * 128.0 - 7.63

import os
FLAG_SCORES_TILED = os.environ.get("K_SCORES_TILED", "1") == "1"
FLAG_EXP_DVE = os.environ.get("K_EXP_DVE", "1") == "1"
FLAG_MUL_GPSIMD = os.environ.get("K_MUL_GPSIMD", "1") == "1"

_CACHED_NC = None


def _build_nc():
    from contextlib import ExitStack

    import concourse.bass as bass
    import concourse.mybir as mybir
    import concourse.tile as tile
    from concourse import bacc

    f32 = mybir.dt.float32
    bf16 = mybir.dt.bfloat16
    i16 = mybir.dt.int16
    AF = mybir.ActivationFunctionType
    ALU = mybir.AluOpType

    nc = bacc.Bacc("TRN2", target_bir_lowering=False, debug=False,
                   num_devices=NCORES)

    # per-core inputs (host pre-sharded / pre-transposed)
    xT = nc.dram_tensor("xT", [C, N], bf16, kind="ExternalInput")
    wqkT = nc.dram_tensor("wqkT", [C, 2 * CG], bf16, kind="ExternalInput")
    wvT = nc.dram_tensor("wvT", [C, CG], bf16, kind="ExternalInput")
    wpT = nc.dram_tensor("wpT", [CG, C], bf16, kind="ExternalInput")
    f16 = mybir.dt.float16
    yp = nc.dram_tensor("yp", [N, C], f16, kind="ExternalOutput")

    with tile.TileContext(nc) as tc:
        with ExitStack() as ctx:
            singles = ctx.enter_context(tc.tile_pool(name="singles", bufs=1))
            tmp = ctx.enter_context(tc.tile_pool(name="tmp", bufs=3))
            ps_big = ctx.enter_context(
                tc.tile_pool(name="ps_big", bufs=3, space="PSUM"))
            ps1 = ctx.enter_context(
                tc.tile_pool(name="ps1", bufs=2, space="PSUM"))
            dscratch = ctx.enter_context(
                tc.tile_pool(name="dscratch", bufs=2, space="DRAM"))

            # ---- persistent SBUF tensors -------------------------------
            xT_sb = singles.tile([P, KT, N], bf16)         # x^T (c on part)
            wqk_sb = singles.tile([P, KT, 2 * CG], bf16)   # q|k weight cols
            wv_sb = singles.tile([P, KT, CG], bf16)
            wp_sb = singles.tile([P, CG // P, C], bf16)
            qT_sb = singles.tile([P, HG // 2, N], bf16)    # q^T (d on part)
            kT_sb = singles.tile([P, HG // 2, N], bf16)    # k^T (d on part)
            v_sb = singles.tile([P, KV_CHUNKS, HG * VB], bf16)
            oT_sb = singles.tile([P, CG // P, N], bf16)    # normalized o^T

            # ---- load inputs ------------------------------------------
            xT_ap = xT.ap().rearrange("(g p) r -> p g r", p=P)
            wqk_ap = wqkT.ap().rearrange("(g p) o -> p g o", p=P)
            # first matmul needs wqk + xT(j, tokens 0:512) — load the first
            # 512-token block of every chunk first (also covers the early
            # v tiles), then the rest j-major on two queues.
            for j in range(KT):
                nc.scalar.dma_start(wqk_sb[:, j, :], wqk_ap[:, j, :])
            for j in range(KT):
                eng = nc.sync if j % 2 == 0 else nc.gpsimd
                eng.dma_start(xT_sb[:, j, 0:512], xT_ap[:, j, 0:512])
            for j in range(KT):
                eng = nc.sync if j % 2 == 0 else nc.gpsimd
                eng.dma_start(xT_sb[:, j, 512:N], xT_ap[:, j, 512:N])
            nc.scalar.dma_start(
                wv_sb[:], wvT.ap().rearrange("(g p) o -> p g o", p=P))
            nc.scalar.dma_start(
                wp_sb[:], wpT.ap().rearrange("(g p) o -> p g o", p=P))
            # whole-tile memset to 1.0; the v copies below overwrite the data
            # columns, leaving the per-head ones columns for the denominator
            nc.vector.memset(v_sb[:], 1.0)
            v_view = v_sb[:].rearrange("p c (h e) -> p c h e", e=VB)

            # ---- q^T / k^T / v projections -----------------------------
            # wqk columns: 0..CG-1 = q channels, CG..2CG-1 = k channels
            # nchunk outer so the first 512-token DMA batch feeds the whole
            # first j-loop; one pts tile per nchunk, rotating.
            def qk_mtile(m, nchunks=range(QT)):
                dst = qT_sb if m < CG // P else kT_sb
                dm = m % (CG // P)
                for nchunk in nchunks:
                    pt = ps_big.tile([P, 1024], f32, tag="sc",
                                     name=f"pts{m}_{nchunk}")
                    for j in range(KT):
                        nc.tensor.matmul(
                            pt[:, 0:512],
                            wqk_sb[:, j, m * P:(m + 1) * P],
                            xT_sb[:, j, nchunk * 512:(nchunk + 1) * 512],
                            start=(j == 0), stop=(j == KT - 1))
                    nc.vector.tensor_copy(
                        out=dst[:, dm, nchunk * 512:(nchunk + 1) * 512],
                        in_=pt[:, 0:512])

            def v_rtile(rt):
                pt = ps_big.tile([P, 1024], f32, tag="sc")
                for j in range(KT):
                    nc.tensor.matmul(
                        pt[:, :CG], xT_sb[:, j, rt * P:(rt + 1) * P],
                        wv_sb[:, j, :], start=(j == 0), stop=(j == KT - 1))
                nc.vector.tensor_copy(
                    out=v_view[:, rt, :, :D],
                    in_=pt[:, :CG].rearrange("p (h d) -> p h d", d=D))

            # emission order minimizes the PE lead-in before the first
            # score matmuls: k/q of pair 0 first (q only needs its first
            # 512-token block), then the rest woven before pair 1's units.
            qk_mtile(2)            # k pair 0 (all 2048 kv)
            qk_mtile(0, [0])       # q pair 0, tokens 0:512 only
            pair0_rest = [(0, 1), (0, 2), (0, 3)]   # (m, nchunk) left for q0
            qk_mtile(3)            # k pair 1
            qk_mtile(1)            # q pair 1

            # PE filler queue: closures emitted one per attention group
            # iteration, each no earlier than `delay` iterations after
            # being enqueued (lets upstream DMA/engine chains complete
            # before the PE hits the dependent matmuls).
            fillers = []           # list of (ready_iteration, closure)
            it_counter = [0]

            def push_filler(fn, delay=0):
                fillers.append((it_counter[0] + delay, fn))

            def pop_filler():
                it_counter[0] += 1
                if fillers and fillers[0][0] <= it_counter[0]:
                    fillers.pop(0)[1]()

            # ---- attention: software-pipelined emission ----------------
            # Units are (pair, qt), qt-major so each 512-row block of the
            # output projection can be emitted as PE filler right after its
            # two units finish.  Within the global stream, the A@V matmuls
            # for group t are emitted AFTER the score matmuls of group t+1:
            # the PE is in-order, so this one-group skew keeps it from
            # stalling on the exp (ACT/DVE) results.
            GROUP = 2  # kv chunks per exp batch (PSUM tile = 2 banks)
            NGRP = KV_CHUNKS // GROUP

            pending_muls = []

            def flush_muls(keep=0):
                while len(pending_muls) > keep:
                    pending_muls.pop(0)()

            def normalize_pair(o_acc_pair, pair, qt):
                # Stage both unnormalized accumulators to SBUF immediately so
                # the PSUM banks free for the next unit's A@V.  The whole
                # chain stays on-chip: ACT evacuates + computes 1/den as
                # exp(-ln(den)) (same ACT table set as the softmax exp),
                # GPSIMD broadcasts the reciprocal across partitions and
                # does the multiply.  No DMA hops, no DVE involvement (its
                # queue is deep with Schraudolph exps).
                for hx, po in ((0, 0), (1, D)):
                    ou = tmp.tile([VB, 512], f32, tag="ou", bufs=4,
                                  name=f"ou{pair}_{qt}_{hx}")
                    nc.scalar.copy(out=ou[:], in_=o_acc_pair[hx][:VB])
                    lnd = tmp.tile([1, 512], f32, tag="lnd",
                                   name=f"lnd{pair}_{qt}_{hx}")
                    nc.scalar.activation(lnd[:], ou[D:D + 1, :], AF.Ln)
                    rec = tmp.tile([1, 512], f32, tag="recs",
                                   name=f"rec{pair}_{qt}_{hx}")
                    nc.scalar.activation(rec[:], lnd[:], AF.Exp, scale=-1.0)
                    bc_sb = tmp.tile([D, 512], f32, tag="bcsb", bufs=4,
                                     name=f"bcsb{pair}_{qt}_{hx}")
                    nc.gpsimd.partition_broadcast(bc_sb[:], rec[:],
                                                  channels=D)

                    def mul(ou=ou, bc_sb=bc_sb, po=po, pair=pair, qt=qt):
                        eng = nc.gpsimd if FLAG_MUL_GPSIMD else nc.vector
                        eng.tensor_mul(
                            out=oT_sb[po:po + D, pair,
                                      qt * 512:(qt + 1) * 512],
                            in0=ou[:D, :], in1=bc_sb[:])
                    pending_muls.append(mul)

            def proj_mt(mt):
                # partial output projection for rows [mt*128, mt*128+128)
                pp = ps_big.tile([P, 1024], f32, tag="sc", name=f"pp{mt}")
                for nh in range(2):
                    for j in range(CG // P):
                        nc.tensor.matmul(
                            pp[:, nh * 512:nh * 512 + 512],
                            oT_sb[:, j, mt * P:(mt + 1) * P],
                            wp_sb[:, j, nh * 512:(nh + 1) * 512],
                            start=(j == 0), stop=(j == CG // P - 1))
                ysb = tmp.tile([P, 1024], f16, tag="ysb", name=f"ysb{mt}")
                nc.scalar.copy(out=ysb[:], in_=pp[:])
                eng = (nc.sync, nc.scalar, nc.gpsimd)[mt % 3]
                eng.dma_start(yp.ap()[mt * P:(mt + 1) * P, :], ysb[:])

            units = [(pair, qt) for qt in range(QT) for pair in range(HG // 2)]
            o_accs_u = {}
            pending = None      # (u, g) whose A@V is not yet emitted

            def emit_av(u, g, exs):
                pair, qt = units[u]
                for i in range(GROUP):
                    r = g * GROUP + i
                    for hx, h in ((0, 2 * pair), (1, 2 * pair + 1)):
                        nc.tensor.matmul(
                            o_accs_u[u][hx][:VB, :],
                            v_sb[:, r, h * VB:(h + 1) * VB],
                            exs[hx][:, i * 512:i * 512 + 512],
                            start=(r == 0), stop=(r == KV_CHUNKS - 1))
                if g == NGRP - 1:
                    normalize_pair(o_accs_u[u], pair, qt)
                    # pair 0: defer its muls one unit so the engine never
                    # waits on the broadcast DMA round-trip.  pair 1: all of
                    # this qt's muls must be EMITTED before the proj fillers
                    # that read oT_sb are (the mul engine absorbs the DMA
                    # wait; proj runs a few groups later anyway).
                    flush_muls(keep=2 if pair == 0 else 0)
                    del o_accs_u[u]
                    if pair == 1:
                        for mt4 in range(4):
                            push_filler(
                                (lambda mt: lambda: proj_mt(mt))(qt * 4 + mt4),
                                delay=6 + 2 * mt4)

            for u, (pair, qt) in enumerate(units):
                qs = slice(qt * 512, (qt + 1) * 512)
                o_accs_u[u] = [ps1.tile([P, 512], f32, tag="ps1",
                                        name=f"oacc{pair}_{qt}_{i}")
                               for i in range(2)]
                if u == 1:
                    for m, nchunk in pair0_rest:
                        push_filler(
                            (lambda a, b: lambda: qk_mtile(a, [b]))(m, nchunk))
                for g in range(NGRP):
                    if u == 0:
                        v_rtile(2 * g)
                        v_rtile(2 * g + 1)
                    else:
                        pop_filler()
                    scs = [ps_big.tile([P, 1024], f32, tag="sc",
                                       name=f"sc{pair}_{qt}_{g}_{i}")
                           for i in range(2)]
                    # 64x64-mode score matmuls: per 512-wide round, 4
                    # concurrent tiles = (head pair) x (kv half of chunk).
                    for i in range(GROUP):
                        r = g * GROUP + i
                        if FLAG_SCORES_TILED:
                            for hx, po in ((0, 0), (1, D)):
                                for kvh in (0, 1):
                                    nc.tensor.matmul(
                                        scs[hx][kvh * 64:kvh * 64 + 64,
                                                i * 512:i * 512 + 512],
                                        kT_sb[po:po + D, pair,
                                              r * P + kvh * 64:
                                              r * P + kvh * 64 + 64],
                                        qT_sb[po:po + D, pair, qs],
                                        start=True, stop=True,
                                        tile_position=(po, kvh * 64))
                        else:
                            for hx, po in ((0, 0), (1, D)):
                                nc.tensor.matmul(
                                    scs[hx][:, i * 512:i * 512 + 512],
                                    kT_sb[po:po + D, pair,
                                          r * P:(r + 1) * P],
                                    qT_sb[po:po + D, pair, qs],
                                    start=True, stop=True)
                    # exp: head A on ACT (table exp), head B on DVE
                    # (Schraudolph bit-trick; per-head softmax cancels its
                    # systematic scale error)
                    ex0 = tmp.tile([P, 1024], bf16, tag="ex", bufs=6,
                                   name=f"ex{pair}_{qt}_{g}_0")
                    nc.scalar.activation(ex0[:], scs[0][:], AF.Exp,
                                         scale=SCALE)
                    if FLAG_EXP_DVE:
                        ex1_i = tmp.tile([P, 1024], i16, tag="exb", bufs=6,
                                         name=f"ex{pair}_{qt}_{g}_1")
                        nc.vector.tensor_scalar(
                            out=ex1_i[:], in0=scs[1][:],
                            scalar1=EXP_A, scalar2=EXP_B,
                            op0=ALU.mult, op1=ALU.add)
                        ex1 = ex1_i[:].bitcast(bf16)
                    else:
                        ex1_t = tmp.tile([P, 1024], bf16, tag="exb2", bufs=6,
                                         name=f"ex{pair}_{qt}_{g}_1")
                        nc.scalar.activation(ex1_t[:], scs[1][:], AF.Exp,
                                             scale=SCALE)
                        ex1 = ex1_t[:]
                    exs = [ex0, ex1]
                    if pending is not None:
                        emit_av(*pending)
                    pending = (u, g, exs)
            emit_av(*pending)
            flush_muls(keep=0)
            while fillers:
                fillers.pop(0)[1]()

    nc.compile()
    return nc


def _host_prep(x, w_qkv, w_proj, b_proj):
    import ml_dtypes
    bf16 = ml_dtypes.bfloat16
    wqkvT = np.ascontiguousarray(w_qkv.T).astype(bf16)   # [C, 3C]
    wpT_full = np.ascontiguousarray(w_proj.T).astype(bf16)  # [C(in), C(out)]
    in_maps = []
    for c in range(NCORES):
        b, g = divmod(c, GROUPS)
        qcols = wqkvT[:, CG * g:CG * (g + 1)]
        kcols = wqkvT[:, C + CG * g:C + CG * (g + 1)]
        vcols = wqkvT[:, 2 * C + CG * g:2 * C + CG * (g + 1)]
        wqk = np.ascontiguousarray(np.concatenate([qcols, kcols], axis=1))
        wv = np.ascontiguousarray(vcols)
        wp = np.ascontiguousarray(wpT_full[CG * g:CG * (g + 1), :])
        xTv = np.ascontiguousarray(x[b].T).astype(bf16)
        in_maps.append({"xT": xTv, "wqkT": wqk, "wvT": wv, "wpT": wp})
    return in_maps


def run(inputs, trace=False, nc=None):
    """Build (or reuse) the program, run on 8 cores, return (y, results)."""
    global _CACHED_NC
    from concourse.bass_utils import run_bass_kernel_spmd
    if nc is None:
        if _CACHED_NC is None:
            _CACHED_NC = _build_nc()
        nc = _CACHED_NC
    in_maps = _host_prep(**inputs)
    res = run_bass_kernel_spmd(nc, in_maps, core_ids=list(range(NCORES)),
                               trace=trace)
    bias = np.asarray(inputs["b_proj"], np.float32)
    out = np.empty((B, N, C), np.float32)
    for b in range(B):
        acc = res.results[b * GROUPS]["yp"].astype(np.float32)
        for g in range(1, GROUPS):
            acc = acc + res.results[b * GROUPS + g]["yp"]
        out[b] = acc + bias
    return out, res


def kernel(x, w_qkv, w_proj, b_proj):
    out, _ = run({"x": np.asarray(x), "w_qkv": np.asarray(w_qkv),
                  "w_proj": np.asarray(w_proj), "b_proj": np.asarray(b_proj)})
    return out


# revision 24
# speedup vs baseline: 1.3244x; 1.3244x over previous
"""Trainium2 Bass kernel for nn_Attention (dense transformer MHA block).

Reference computation (fp32):
    qkv = x @ w_qkv.T            # [B,N,3C]
    q,k,v per head; scores = q k^T / sqrt(D); attn = softmax(scores)
    o = attn @ v;  y = o @ w_proj.T + b_proj

Sharding over 8 NeuronCores (data-parallel over batch x tensor-parallel over
heads): core c -> (batch b = c//4, head group g = c%4, heads 4g..4g+3).
Each core computes q/k/v for its 4 heads over the full 2048-token sequence,
runs attention locally, and multiplies by its row-slice of w_proj, producing
a PARTIAL output [2048, 1024].  The 4 partials per batch are summed on the
host (numpy) together with the bias — no device collectives.

Perf structure (v2):
  - score matmuls run in 64x64 array-tiling mode: 4 concurrent tiles
    (2 heads x 2 kv-halves) per 512-wide round, ~2x faster than the
    untiled 64-contraction matmuls.
  - exp is split across engines: head A of each pair uses the ACT table
    exp; head B uses a Schraudolph bit-trick exp on the DVE
    (y_i16 = round(s*A + B) bitcast as bf16 == exp(s*scale) within ~4%).
    The per-head softmax normalization cancels each head's systematic
    scale error.
  - A@V keeps the ones-column trick (V gets a 65th column of ones so the
    same matmul accumulates the softmax denominator) - provably optimal
    since any separate denominator matmul would re-stream all exp
    columns through the PE.
  - x DMA is j(contraction-chunk)-major across two queues, with the
    first 512-token block of all chunks first so both the n-outer qk
    matmuls and the early v tiles never stall.
  - output projection for each 512-row block is emitted as soon as its
    two attention units finish (PE filler), normalize multiplies run on
    GPSIMD, PSUM->SBUF output copies on ACT.
"""

import numpy as np

B, N, C = 2, 2048, 1024
H, D = 16, 64
NCORES = 8
GROUPS = 4              # head groups (tensor-parallel)
HG = H // GROUPS        # 4 heads per core
CG = HG * D             # 256 channels per core
P = 128
KT = C // P             # 8 contraction subtiles for C=1024
KV_CHUNKS = N // P      # 16 key/value chunks of 128 rows
QT = N // 512           # 4 query tiles of 512
VB = D + 1              # v block width incl. ones column (65)
SCALE = 1.0 / float(np.sqrt(D))
# Schraudolph exp constants (bf16 bit-trick on DVE): for scores s (pre-scale),
# exp(s*SCALE) ~= bitcast_bf16(int16(s*EXP_A + EXP_B)).  The -7.63 centers the
# sawtooth approximation error; the per-head softmax cancels the global scale.
EXP_A = 128.0 * 1.4426950408889634 * SCALE
EXP_B = 127.0 * 128.0 - 7.63

import os
FLAG_SCORES_TILED = os.environ.get("K_SCORES_TILED", "1") == "1"
FLAG_EXP_DVE = os.environ.get("K_EXP_DVE", "1") == "1"
FLAG_MUL_GPSIMD = os.environ.get("K_MUL_GPSIMD", "1") == "1"

_CACHED_NC = None


def _build_nc():
    from contextlib import ExitStack

    import concourse.bass as bass
    import concourse.mybir as mybir
    import concourse.tile as tile
    from concourse import bacc

    f32 = mybir.dt.float32
    bf16 = mybir.dt.bfloat16
    i16 = mybir.dt.int16
    AF = mybir.ActivationFunctionType
    ALU = mybir.AluOpType

    nc = bacc.Bacc("TRN2", target_bir_lowering=False, debug=False,
                   num_devices=NCORES)

    # per-core inputs (host pre-sharded / pre-transposed)
    xT = nc.dram_tensor("xT", [C, N], bf16, kind="ExternalInput")
    wqkT = nc.dram_tensor("wqkT", [C, 2 * CG], bf16, kind="ExternalInput")
    wvT = nc.dram_tensor("wvT", [C, CG], bf16, kind="ExternalInput")
    wpT = nc.dram_tensor("wpT", [CG, C], bf16, kind="ExternalInput")
    f16 = mybir.dt.float16
    yp = nc.dram_tensor("yp", [N, C], f16, kind="ExternalOutput")

    with tile.TileContext(nc) as tc:
        with ExitStack() as ctx:
            singles = ctx.enter_context(tc.tile_pool(name="singles", bufs=1))
            tmp = ctx.enter_context(tc.tile_pool(name="tmp", bufs=3))
            ps_big = ctx.enter_context(
                tc.tile_pool(name="ps_big", bufs=3, space="PSUM"))
            ps1 = ctx.enter_context(
                tc.tile_pool(name="ps1", bufs=2, space="PSUM"))
            dscratch = ctx.enter_context(
                tc.tile_pool(name="dscratch", bufs=2, space="DRAM"))

            # ---- persistent SBUF tensors -------------------------------
            xT_sb = singles.tile([P, KT, N], bf16)         # x^T (c on part)
            wqk_sb = singles.tile([P, KT, 2 * CG], bf16)   # q|k weight cols
            wv_sb = singles.tile([P, KT, CG], bf16)
            wp_sb = singles.tile([P, CG // P, C], bf16)
            qT_sb = singles.tile([P, HG // 2, N], bf16)    # q^T (d on part)
            kT_sb = singles.tile([P, HG // 2, N], bf16)    # k^T (d on part)
            v_sb = singles.tile([P, KV_CHUNKS, HG * VB], bf16)
            oT_sb = singles.tile([P, CG // P, N], bf16)    # normalized o^T

            # ---- load inputs ------------------------------------------
            xT_ap = xT.ap().rearrange("(g p) r -> p g r", p=P)
            wqk_ap = wqkT.ap().rearrange("(g p) o -> p g o", p=P)
            # first matmul needs wqk + xT(j, tokens 0:512) — load the first
            # 512-token block of every chunk first (also covers the early
            # v tiles), then the rest j-major on two queues.
            for j in range(KT):
                nc.scalar.dma_start(wqk_sb[:, j, :], wqk_ap[:, j, :])
            for j in range(KT):
                eng = nc.sync if j % 2 == 0 else nc.gpsimd
                eng.dma_start(xT_sb[:, j, 0:512], xT_ap[:, j, 0:512])
            for j in range(KT):
                eng = nc.sync if j % 2 == 0 else nc.gpsimd
                eng.dma_start(xT_sb[:, j, 512:N], xT_ap[:, j, 512:N])
            nc.scalar.dma_start(
                wv_sb[:], wvT.ap().rearrange("(g p) o -> p g o", p=P))
            nc.scalar.dma_start(
                wp_sb[:], wpT.ap().rearrange("(g p) o -> p g o", p=P))
            # whole-tile memset to 1.0; the v copies below overwrite the data
            # columns, leaving the per-head ones columns for the denominator
            nc.vector.memset(v_sb[:], 1.0)
            v_view = v_sb[:].rearrange("p c (h e) -> p c h e", e=VB)

            # ---- q^T / k^T / v projections -----------------------------
            # wqk columns: 0..CG-1 = q channels, CG..2CG-1 = k channels
            # nchunk outer so the first 512-token DMA batch feeds the whole
            # first j-loop; one pts tile per nchunk, rotating.
            def qk_mtile(m, nchunks=range(QT)):
                dst = qT_sb if m < CG // P else kT_sb
                dm = m % (CG // P)
                for nchunk in nchunks:
                    pt = ps_big.tile([P, 1024], f32, tag="sc",
                                     name=f"pts{m}_{nchunk}")
                    for j in range(KT):
                        nc.tensor.matmul(
                            pt[:, 0:512],
                            wqk_sb[:, j, m * P:(m + 1) * P],
                            xT_sb[:, j, nchunk * 512:(nchunk + 1) * 512],
                            start=(j == 0), stop=(j == KT - 1))
                    nc.vector.tensor_copy(
                        out=dst[:, dm, nchunk * 512:(nchunk + 1) * 512],
                        in_=pt[:, 0:512])

            def v_rtile(rt):
                pt = ps_big.tile([P, 1024], f32, tag="sc")
                for j in range(KT):
                    nc.tensor.matmul(
                        pt[:, :CG], xT_sb[:, j, rt * P:(rt + 1) * P],
                        wv_sb[:, j, :], start=(j == 0), stop=(j == KT - 1))
                nc.vector.tensor_copy(
                    out=v_view[:, rt, :, :D],
                    in_=pt[:, :CG].rearrange("p (h d) -> p h d", d=D))

            # emission order minimizes the PE lead-in before the first
            # score matmuls: k/q of pair 0 first (q only needs its first
            # 512-token block), then the rest woven before pair 1's units.
            qk_mtile(2)            # k pair 0 (all 2048 kv)
            qk_mtile(0, [0])       # q pair 0, tokens 0:512 only
            pair0_rest = [(0, 1), (0, 2), (0, 3)]   # (m, nchunk) left for q0
            qk_mtile(3)            # k pair 1
            qk_mtile(1)            # q pair 1

            # PE filler queue: closures emitted one per attention group
            # iteration, each no earlier than `delay` iterations after
            # being enqueued (lets upstream DMA/engine chains complete
            # before the PE hits the dependent matmuls).
            fillers = []           # list of (ready_iteration, closure)
            it_counter = [0]

            def push_filler(fn, delay=0):
                fillers.append((it_counter[0] + delay, fn))

            def pop_filler():
                it_counter[0] += 1
                if fillers and fillers[0][0] <= it_counter[0]:
                    fillers.pop(0)[1]()

            # ---- attention: software-pipelined emission ----------------
            # Units are (pair, qt), qt-major so each 512-row block of the
            # output projection can be emitted as PE filler right after its
            # two units finish.  Within the global stream, the A@V matmuls
            # for group t are emitted AFTER the score matmuls of group t+1:
            # the PE is in-order, so this one-group skew keeps it from
            # stalling on the exp (ACT/DVE) results.
            GROUP = 2  # kv chunks per exp batch (PSUM tile = 2 banks)
            NGRP = KV_CHUNKS // GROUP

            pending_muls = []

            def flush_muls(keep=0):
                while len(pending_muls) > keep:
                    pending_muls.pop(0)()

            def normalize_pair(o_acc_pair, pair, qt):
                # Stage both unnormalized accumulators to SBUF immediately so
                # the PSUM banks free for the next unit's A@V.  The whole
                # chain stays on-chip: ACT evacuates + computes 1/den as
                # exp(-ln(den)) (same ACT table set as the softmax exp),
                # GPSIMD broadcasts the reciprocal across partitions and
                # does the multiply.  No DMA hops, no DVE involvement (its
                # queue is deep with Schraudolph exps).
                for hx, po in ((0, 0), (1, D)):
                    # o_acc rows: 0..63 = data, 64 = denominator (ones col)
                    ou = tmp.tile([D, 512], f32, tag="ou", bufs=4,
                                  name=f"ou{pair}_{qt}_{hx}")
                    nc.scalar.copy(out=ou[:], in_=o_acc_pair[hx][:D])
                    den_t = tmp.tile([1, 512], f32, tag="dent", bufs=4,
                                     name=f"dent{pair}_{qt}_{hx}")
                    nc.scalar.copy(out=den_t[:],
                                   in_=o_acc_pair[hx][D:D + 1])
                    bc_sb = tmp.tile([D, 512], f32, tag="bcsb", bufs=4,
                                     name=f"bcsb{pair}_{qt}_{hx}")
                    nc.gpsimd.partition_broadcast(bc_sb[:], den_t[:],
                                                  channels=D)
                    rec_bc = tmp.tile([D, 512], f32, tag="recbc", bufs=4,
                                      name=f"recbc{pair}_{qt}_{hx}")
                    nc.vector.reciprocal(out=rec_bc[:], in_=bc_sb[:])

                    def mul(ou=ou, rec_bc=rec_bc, po=po, pair=pair, qt=qt):
                        eng = nc.gpsimd if FLAG_MUL_GPSIMD else nc.vector
                        eng.tensor_mul(
                            out=oT_sb[po:po + D, pair,
                                      qt * 512:(qt + 1) * 512],
                            in0=ou[:], in1=rec_bc[:])
                    pending_muls.append(mul)

            def proj_mt(mt):
                # partial output projection for rows [mt*128, mt*128+128)
                pp = ps_big.tile([P, 1024], f32, tag="sc", name=f"pp{mt}")
                for nh in range(2):
                    for j in range(CG // P):
                        nc.tensor.matmul(
                            pp[:, nh * 512:nh * 512 + 512],
                            oT_sb[:, j, mt * P:(mt + 1) * P],
                            wp_sb[:, j, nh * 512:(nh + 1) * 512],
                            start=(j == 0), stop=(j == CG // P - 1))
                ysb = tmp.tile([P, 1024], f16, tag="ysb", name=f"ysb{mt}")
                nc.scalar.copy(out=ysb[:], in_=pp[:])
                eng = (nc.sync, nc.scalar, nc.gpsimd)[mt % 3]
                eng.dma_start(yp.ap()[mt * P:(mt + 1) * P, :], ysb[:])

            units = [(pair, qt) for qt in range(QT) for pair in range(HG // 2)]
            o_accs_u = {}
            pending = None      # (u, g) whose A@V is not yet emitted

            def emit_av(u, g, exs):
                pair, qt = units[u]
                for i in range(GROUP):
                    r = g * GROUP + i
                    for hx, h in ((0, 2 * pair), (1, 2 * pair + 1)):
                        nc.tensor.matmul(
                            o_accs_u[u][hx][:VB, :],
                            v_sb[:, r, h * VB:(h + 1) * VB],
                            exs[hx][:, i * 512:i * 512 + 512],
                            start=(r == 0), stop=(r == KV_CHUNKS - 1))
                if g == NGRP - 1:
                    normalize_pair(o_accs_u[u], pair, qt)
                    # pair 0: defer its muls one unit so the engine never
                    # waits on the broadcast DMA round-trip.  pair 1: all of
                    # this qt's muls must be EMITTED before the proj fillers
                    # that read oT_sb are (the mul engine absorbs the DMA
                    # wait; proj runs a few groups later anyway).
                    flush_muls(keep=2 if pair == 0 else 0)
                    del o_accs_u[u]
                    if pair == 1:
                        for mt4 in range(4):
                            push_filler(
                                (lambda mt: lambda: proj_mt(mt))(qt * 4 + mt4),
                                delay=6 + 2 * mt4)

            for u, (pair, qt) in enumerate(units):
                qs = slice(qt * 512, (qt + 1) * 512)
                o_accs_u[u] = [ps1.tile([P, 512], f32, tag="ps1",
                                        name=f"oacc{pair}_{qt}_{i}")
                               for i in range(2)]
                if u == 1:
                    for m, nchunk in pair0_rest:
                        push_filler(
                            (lambda a, b: lambda: qk_mtile(a, [b]))(m, nchunk))
                for g in range(NGRP):
                    if u == 0:
                        v_rtile(2 * g)
                        v_rtile(2 * g + 1)
                    else:
                        pop_filler()
                    scs = [ps_big.tile([P, 1024], f32, tag="sc",
                                       name=f"sc{pair}_{qt}_{g}_{i}")
                           for i in range(2)]
                    # 64x64-mode score matmuls: per 512-wide round, 4
                    # concurrent tiles = (head pair) x (kv half of chunk).
                    for i in range(GROUP):
                        r = g * GROUP + i
                        if FLAG_SCORES_TILED:
                            for hx, po in ((0, 0), (1, D)):
                                for kvh in (0, 1):
                                    nc.tensor.matmul(
                                        scs[hx][kvh * 64:kvh * 64 + 64,
                                                i * 512:i * 512 + 512],
                                        kT_sb[po:po + D, pair,
                                              r * P + kvh * 64:
                                              r * P + kvh * 64 + 64],
                                        qT_sb[po:po + D, pair, qs],
                                        start=True, stop=True,
                                        tile_position=(po, kvh * 64))
                        else:
                            for hx, po in ((0, 0), (1, D)):
                                nc.tensor.matmul(
                                    scs[hx][:, i * 512:i * 512 + 512],
                                    kT_sb[po:po + D, pair,
                                          r * P:(r + 1) * P],
                                    qT_sb[po:po + D, pair, qs],
                                    start=True, stop=True)
                    # exp: head A on ACT (table exp), head B on DVE
                    # (Schraudolph bit-trick; per-head softmax cancels its
                    # systematic scale error)
                    ex0 = tmp.tile([P, 1024], bf16, tag="ex", bufs=6,
                                   name=f"ex{pair}_{qt}_{g}_0")
                    nc.scalar.activation(ex0[:], scs[0][:], AF.Exp,
                                         scale=SCALE)
                    if FLAG_EXP_DVE:
                        ex1_i = tmp.tile([P, 1024], i16, tag="exb", bufs=6,
                                         name=f"ex{pair}_{qt}_{g}_1")
                        nc.vector.tensor_scalar(
                            out=ex1_i[:], in0=scs[1][:],
                            scalar1=EXP_A, scalar2=EXP_B,
                            op0=ALU.mult, op1=ALU.add)
                        ex1 = ex1_i[:].bitcast(bf16)
                    else:
                        ex1_t = tmp.tile([P, 1024], bf16, tag="exb2", bufs=6,
                                         name=f"ex{pair}_{qt}_{g}_1")
                        nc.scalar.activation(ex1_t[:], scs[1][:], AF.Exp,
                                             scale=SCALE)
                        ex1 = ex1_t[:]
                    exs = [ex0, ex1]
                    if pending is not None:
                        emit_av(*pending)
                    pending = (u, g, exs)
            emit_av(*pending)
            flush_muls(keep=0)
            while fillers:
                fillers.pop(0)[1]()

    nc.compile()
    return nc


def _host_prep(x, w_qkv, w_proj, b_proj):
    import ml_dtypes
    bf16 = ml_dtypes.bfloat16
    wqkvT = np.ascontiguousarray(w_qkv.T).astype(bf16)   # [C, 3C]
    wpT_full = np.ascontiguousarray(w_proj.T).astype(bf16)  # [C(in), C(out)]
    in_maps = []
    for c in range(NCORES):
        b, g = divmod(c, GROUPS)
        qcols = wqkvT[:, CG * g:CG * (g + 1)]
        kcols = wqkvT[:, C + CG * g:C + CG * (g + 1)]
        vcols = wqkvT[:, 2 * C + CG * g:2 * C + CG * (g + 1)]
        wqk = np.ascontiguousarray(np.concatenate([qcols, kcols], axis=1))
        wv = np.ascontiguousarray(vcols)
        wp = np.ascontiguousarray(wpT_full[CG * g:CG * (g + 1), :])
        xTv = np.ascontiguousarray(x[b].T).astype(bf16)
        in_maps.append({"xT": xTv, "wqkT": wqk, "wvT": wv, "wpT": wp})
    return in_maps


def run(inputs, trace=False, nc=None):
    """Build (or reuse) the program, run on 8 cores, return (y, results)."""
    global _CACHED_NC
    from concourse.bass_utils import run_bass_kernel_spmd
    if nc is None:
        if _CACHED_NC is None:
            _CACHED_NC = _build_nc()
        nc = _CACHED_NC
    in_maps = _host_prep(**inputs)
    res = run_bass_kernel_spmd(nc, in_maps, core_ids=list(range(NCORES)),
                               trace=trace)
    bias = np.asarray(inputs["b_proj"], np.float32)
    out = np.empty((B, N, C), np.float32)
    for b in range(B):
        acc = res.results[b * GROUPS]["yp"].astype(np.float32)
        for g in range(1, GROUPS):
            acc = acc + res.results[b * GROUPS + g]["yp"]
        out[b] = acc + bias
    return out, res


def kernel(x, w_qkv, w_proj, b_proj):
    out, _ = run({"x": np.asarray(x), "w_qkv": np.asarray(w_qkv),
                  "w_proj": np.asarray(w_proj), "b_proj": np.asarray(b_proj)})
    return out


# revision 25
# speedup vs baseline: 1.4977x; 1.1309x over previous
"""Trainium2 Bass kernel for nn_Attention (dense transformer MHA block).

Reference computation (fp32):
    qkv = x @ w_qkv.T            # [B,N,3C]
    q,k,v per head; scores = q k^T / sqrt(D); attn = softmax(scores)
    o = attn @ v;  y = o @ w_proj.T + b_proj

Sharding over 8 NeuronCores (data-parallel over batch x tensor-parallel over
heads): core c -> (batch b = c//4, head group g = c%4, heads 4g..4g+3).
Each core computes q/k/v for its 4 heads over the full 2048-token sequence,
runs attention locally, and multiplies by its row-slice of w_proj, producing
a PARTIAL output [2048, 1024].  The 4 partials per batch are summed on the
host (numpy) together with the bias — no device collectives.

Perf structure (v2):
  - score matmuls run in 64x64 array-tiling mode: 4 concurrent tiles
    (2 heads x 2 kv-halves) per 512-wide round, ~2x faster than the
    untiled 64-contraction matmuls.
  - exp is split across engines: head A of each pair uses the ACT table
    exp; head B uses a Schraudolph bit-trick exp on the DVE
    (y_i16 = round(s*A + B) bitcast as bf16 == exp(s*scale) within ~4%).
    The per-head softmax normalization cancels each head's systematic
    scale error.
  - A@V keeps the ones-column trick (V gets a 65th column of ones so the
    same matmul accumulates the softmax denominator) - provably optimal
    since any separate denominator matmul would re-stream all exp
    columns through the PE.
  - x DMA is j(contraction-chunk)-major across two queues, with the
    first 512-token block of all chunks first so both the n-outer qk
    matmuls and the early v tiles never stall.
  - output projection for each 512-row block is emitted as soon as its
    two attention units finish (PE filler), normalize multiplies run on
    GPSIMD, PSUM->SBUF output copies on ACT.
"""

import numpy as np

B, N, C = 2, 2048, 1024
H, D = 16, 64
NCORES = 8
GROUPS = 4              # head groups (tensor-parallel)
HG = H // GROUPS        # 4 heads per core
CG = HG * D             # 256 channels per core
P = 128
KT = C // P             # 8 contraction subtiles for C=1024
KV_CHUNKS = N // P      # 16 key/value chunks of 128 rows
QT = N // 512           # 4 query tiles of 512
VB = D + 1              # v block width incl. ones column (65)
SCALE = 1.0 / float(np.sqrt(D))
# Schraudolph exp constants (bf16 bit-trick on DVE): for scores s (pre-scale),
# exp(s*SCALE) ~= bitcast_bf16(int16(s*EXP_A + EXP_B)).  The -7.63 centers the
# sawtooth approximation error; the per-head softmax cancels the global scale.
EXP_A = 128.0 * 1.4426950408889634 * SCALE
EXP_B = 127.0 * 128.0 - 7.63

import os
FLAG_SCORES_TILED = os.environ.get("K_SCORES_TILED", "1") == "1"
FLAG_EXP_DVE = os.environ.get("K_EXP_DVE", "1") == "1"
FLAG_MUL_GPSIMD = os.environ.get("K_MUL_GPSIMD", "1") == "1"

_CACHED_NC = None


def _build_nc():
    from contextlib import ExitStack

    import concourse.bass as bass
    import concourse.mybir as mybir
    import concourse.tile as tile
    from concourse import bacc

    f32 = mybir.dt.float32
    bf16 = mybir.dt.bfloat16
    i16 = mybir.dt.int16
    AF = mybir.ActivationFunctionType
    ALU = mybir.AluOpType

    nc = bacc.Bacc("TRN2", target_bir_lowering=False, debug=False,
                   num_devices=NCORES)

    # per-core inputs (host pre-sharded / pre-transposed)
    xT = nc.dram_tensor("xT", [C, N], bf16, kind="ExternalInput")
    wqkT = nc.dram_tensor("wqkT", [C, 2 * CG], bf16, kind="ExternalInput")
    wvT = nc.dram_tensor("wvT", [C, CG], bf16, kind="ExternalInput")
    wpT = nc.dram_tensor("wpT", [CG, C], bf16, kind="ExternalInput")
    f16 = mybir.dt.float16
    yp = nc.dram_tensor("yp", [N, C], f16, kind="ExternalOutput")

    with tile.TileContext(nc) as tc:
        with ExitStack() as ctx:
            singles = ctx.enter_context(tc.tile_pool(name="singles", bufs=1))
            tmp = ctx.enter_context(tc.tile_pool(name="tmp", bufs=3))
            ps_big = ctx.enter_context(
                tc.tile_pool(name="ps_big", bufs=3, space="PSUM"))
            ps1 = ctx.enter_context(
                tc.tile_pool(name="ps1", bufs=2, space="PSUM"))
            dscratch = ctx.enter_context(
                tc.tile_pool(name="dscratch", bufs=2, space="DRAM"))

            # ---- persistent SBUF tensors -------------------------------
            xT_sb = singles.tile([P, KT, N], bf16)         # x^T (c on part)
            wqk_sb = singles.tile([P, KT, 2 * CG], bf16)   # q|k weight cols
            wv_sb = singles.tile([P, KT, CG], bf16)
            wp_sb = singles.tile([P, CG // P, C], bf16)
            qT_sb = singles.tile([P, HG // 2, N], bf16)    # q^T (d on part)
            kT_sb = singles.tile([P, HG // 2, N], bf16)    # k^T (d on part)
            v_sb = singles.tile([P, KV_CHUNKS, HG * VB], bf16)
            oT_sb = singles.tile([P, CG // P, N], bf16)    # normalized o^T

            # ---- load inputs ------------------------------------------
            xT_ap = xT.ap().rearrange("(g p) r -> p g r", p=P)
            wqk_ap = wqkT.ap().rearrange("(g p) o -> p g o", p=P)
            # first matmul needs wqk + xT(j, tokens 0:512) — load the first
            # 512-token block of every chunk first (also covers the early
            # v tiles), then the rest j-major on two queues.
            for j in range(KT):
                nc.scalar.dma_start(wqk_sb[:, j, :], wqk_ap[:, j, :])
            for j in range(KT):
                eng = nc.sync if j % 2 == 0 else nc.gpsimd
                eng.dma_start(xT_sb[:, j, 0:512], xT_ap[:, j, 0:512])
            for j in range(KT):
                eng = nc.sync if j % 2 == 0 else nc.gpsimd
                eng.dma_start(xT_sb[:, j, 512:N], xT_ap[:, j, 512:N])
            nc.scalar.dma_start(
                wv_sb[:], wvT.ap().rearrange("(g p) o -> p g o", p=P))
            nc.scalar.dma_start(
                wp_sb[:], wpT.ap().rearrange("(g p) o -> p g o", p=P))
            # whole-tile memset to 1.0; the v copies below overwrite the data
            # columns, leaving the per-head ones columns for the denominator
            nc.vector.memset(v_sb[:], 1.0)
            v_view = v_sb[:].rearrange("p c (h e) -> p c h e", e=VB)

            # ---- q^T / k^T / v projections -----------------------------
            # wqk columns: 0..CG-1 = q channels, CG..2CG-1 = k channels
            # nchunk outer so the first 512-token DMA batch feeds the whole
            # first j-loop; one pts tile per nchunk, rotating.
            def qk_mtile(m, nchunks=range(QT)):
                dst = qT_sb if m < CG // P else kT_sb
                dm = m % (CG // P)
                for nchunk in nchunks:
                    pt = ps_big.tile([P, 1024], f32, tag="sc",
                                     name=f"pts{m}_{nchunk}")
                    for j in range(KT):
                        nc.tensor.matmul(
                            pt[:, 0:512],
                            wqk_sb[:, j, m * P:(m + 1) * P],
                            xT_sb[:, j, nchunk * 512:(nchunk + 1) * 512],
                            start=(j == 0), stop=(j == KT - 1))
                    nc.vector.tensor_copy(
                        out=dst[:, dm, nchunk * 512:(nchunk + 1) * 512],
                        in_=pt[:, 0:512])

            def v_rtile(rt):
                pt = ps_big.tile([P, 1024], f32, tag="sc")
                for j in range(KT):
                    nc.tensor.matmul(
                        pt[:, :CG], xT_sb[:, j, rt * P:(rt + 1) * P],
                        wv_sb[:, j, :], start=(j == 0), stop=(j == KT - 1))
                nc.vector.tensor_copy(
                    out=v_view[:, rt, :, :D],
                    in_=pt[:, :CG].rearrange("p (h d) -> p h d", d=D))

            # emission order minimizes the PE lead-in before the first
            # score matmuls: k/q of pair 0 first (q only needs its first
            # 512-token block), then the rest woven before pair 1's units.
            qk_mtile(2)            # k pair 0 (all 2048 kv)
            qk_mtile(0, [0])       # q pair 0, tokens 0:512 only
            pair0_rest = [(0, 1), (0, 2), (0, 3)]   # (m, nchunk) left for q0
            qk_mtile(3)            # k pair 1
            qk_mtile(1)            # q pair 1

            # PE filler queue: closures emitted one per attention group
            # iteration, each no earlier than `delay` iterations after
            # being enqueued (lets upstream DMA/engine chains complete
            # before the PE hits the dependent matmuls).
            fillers = []           # list of (ready_iteration, closure)
            it_counter = [0]

            def push_filler(fn, delay=0):
                fillers.append((it_counter[0] + delay, fn))

            def pop_filler():
                it_counter[0] += 1
                if fillers and fillers[0][0] <= it_counter[0]:
                    fillers.pop(0)[1]()

            # ---- attention: software-pipelined emission ----------------
            # Units are (pair, qt), qt-major so each 512-row block of the
            # output projection can be emitted as PE filler right after its
            # two units finish.  Within the global stream, the A@V matmuls
            # for group t are emitted AFTER the score matmuls of group t+1:
            # the PE is in-order, so this one-group skew keeps it from
            # stalling on the exp (ACT/DVE) results.
            GROUP = 2  # kv chunks per exp batch (PSUM tile = 2 banks)
            NGRP = KV_CHUNKS // GROUP

            pending_muls = []

            def flush_muls(keep=0):
                while len(pending_muls) > keep:
                    pending_muls.pop(0)()

            def normalize_pair(o_acc_pair, pair, qt):
                # Stage both unnormalized accumulators to SBUF immediately so
                # the PSUM banks free for the next unit's A@V.  The whole
                # chain stays on-chip: ACT evacuates + computes 1/den as
                # exp(-ln(den)) (same ACT table set as the softmax exp),
                # GPSIMD broadcasts the reciprocal across partitions and
                # does the multiply.  No DMA hops, no DVE involvement (its
                # queue is deep with Schraudolph exps).
                for hx, po in ((0, 0), (1, D)):
                    # o_acc rows: 0..63 = data, 64 = denominator (ones col)
                    ou = tmp.tile([D, 512], f32, tag="ou", bufs=4,
                                  name=f"ou{pair}_{qt}_{hx}")
                    nc.scalar.copy(out=ou[:], in_=o_acc_pair[hx][:D])
                    den_t = tmp.tile([1, 512], f32, tag="dent", bufs=4,
                                     name=f"dent{pair}_{qt}_{hx}")
                    nc.scalar.copy(out=den_t[:],
                                   in_=o_acc_pair[hx][D:D + 1])
                    bc_sb = tmp.tile([D, 512], f32, tag="bcsb", bufs=4,
                                     name=f"bcsb{pair}_{qt}_{hx}")
                    nc.gpsimd.partition_broadcast(bc_sb[:], den_t[:],
                                                  channels=D)
                    rec_bc = tmp.tile([D, 512], f32, tag="recbc", bufs=4,
                                      name=f"recbc{pair}_{qt}_{hx}")
                    nc.vector.reciprocal_approx_fast(out=rec_bc[:],
                                                     in_=bc_sb[:])

                    def mul(ou=ou, rec_bc=rec_bc, po=po, pair=pair, qt=qt):
                        eng = nc.gpsimd if FLAG_MUL_GPSIMD else nc.vector
                        eng.tensor_mul(
                            out=oT_sb[po:po + D, pair,
                                      qt * 512:(qt + 1) * 512],
                            in0=ou[:], in1=rec_bc[:])
                    pending_muls.append(mul)

            def proj_mt(mt):
                # partial output projection for rows [mt*128, mt*128+128)
                pp = ps_big.tile([P, 1024], f32, tag="sc", name=f"pp{mt}")
                for nh in range(2):
                    for j in range(CG // P):
                        nc.tensor.matmul(
                            pp[:, nh * 512:nh * 512 + 512],
                            oT_sb[:, j, mt * P:(mt + 1) * P],
                            wp_sb[:, j, nh * 512:(nh + 1) * 512],
                            start=(j == 0), stop=(j == CG // P - 1))
                ysb = tmp.tile([P, 1024], f16, tag="ysb", name=f"ysb{mt}")
                nc.scalar.copy(out=ysb[:], in_=pp[:])
                eng = (nc.sync, nc.scalar, nc.gpsimd)[mt % 3]
                eng.dma_start(yp.ap()[mt * P:(mt + 1) * P, :], ysb[:])

            units = [(pair, qt) for qt in range(QT) for pair in range(HG // 2)]
            o_accs_u = {}
            pending = None      # (u, g) whose A@V is not yet emitted

            def emit_av(u, g, exs):
                pair, qt = units[u]
                for i in range(GROUP):
                    r = g * GROUP + i
                    for hx, h in ((0, 2 * pair), (1, 2 * pair + 1)):
                        nc.tensor.matmul(
                            o_accs_u[u][hx][:VB, :],
                            v_sb[:, r, h * VB:(h + 1) * VB],
                            exs[hx][:, i * 512:i * 512 + 512],
                            start=(r == 0), stop=(r == KV_CHUNKS - 1))
                if g == NGRP - 1:
                    normalize_pair(o_accs_u[u], pair, qt)
                    # pair 0: defer its muls one unit so the engine never
                    # waits on the broadcast DMA round-trip.  pair 1: all of
                    # this qt's muls must be EMITTED before the proj fillers
                    # that read oT_sb are (the mul engine absorbs the DMA
                    # wait; proj runs a few groups later anyway).
                    flush_muls(keep=2 if pair == 0 else 0)
                    del o_accs_u[u]
                    if pair == 1:
                        for mt4 in range(4):
                            push_filler(
                                (lambda mt: lambda: proj_mt(mt))(qt * 4 + mt4),
                                delay=6 + 2 * mt4)

            for u, (pair, qt) in enumerate(units):
                qs = slice(qt * 512, (qt + 1) * 512)
                o_accs_u[u] = [ps1.tile([P, 512], f32, tag="ps1",
                                        name=f"oacc{pair}_{qt}_{i}")
                               for i in range(2)]
                if u == 1:
                    for m, nchunk in pair0_rest:
                        push_filler(
                            (lambda a, b: lambda: qk_mtile(a, [b]))(m, nchunk))
                for g in range(NGRP):
                    if u == 0:
                        v_rtile(2 * g)
                        v_rtile(2 * g + 1)
                    else:
                        pop_filler()
                    scs = [ps_big.tile([P, 1024], f32, tag="sc",
                                       name=f"sc{pair}_{qt}_{g}_{i}")
                           for i in range(2)]
                    # 64x64-mode score matmuls: per 512-wide round, 4
                    # concurrent tiles = (head pair) x (kv half of chunk).
                    for i in range(GROUP):
                        r = g * GROUP + i
                        if FLAG_SCORES_TILED:
                            for hx, po in ((0, 0), (1, D)):
                                for kvh in (0, 1):
                                    nc.tensor.matmul(
                                        scs[hx][kvh * 64:kvh * 64 + 64,
                                                i * 512:i * 512 + 512],
                                        kT_sb[po:po + D, pair,
                                              r * P + kvh * 64:
                                              r * P + kvh * 64 + 64],
                                        qT_sb[po:po + D, pair, qs],
                                        start=True, stop=True,
                                        tile_position=(po, kvh * 64))
                        else:
                            for hx, po in ((0, 0), (1, D)):
                                nc.tensor.matmul(
                                    scs[hx][:, i * 512:i * 512 + 512],
                                    kT_sb[po:po + D, pair,
                                          r * P:(r + 1) * P],
                                    qT_sb[po:po + D, pair, qs],
                                    start=True, stop=True)
                    # exp: head A on ACT (table exp), head B on DVE
                    # (Schraudolph bit-trick; per-head softmax cancels its
                    # systematic scale error)
                    ex0 = tmp.tile([P, 1024], bf16, tag="ex", bufs=6,
                                   name=f"ex{pair}_{qt}_{g}_0")
                    nc.scalar.activation(ex0[:], scs[0][:], AF.Exp,
                                         scale=SCALE)
                    if FLAG_EXP_DVE:
                        ex1_i = tmp.tile([P, 1024], i16, tag="exb", bufs=6,
                                         name=f"ex{pair}_{qt}_{g}_1")
                        nc.vector.tensor_scalar(
                            out=ex1_i[:], in0=scs[1][:],
                            scalar1=EXP_A, scalar2=EXP_B,
                            op0=ALU.mult, op1=ALU.add)
                        ex1 = ex1_i[:].bitcast(bf16)
                    else:
                        ex1_t = tmp.tile([P, 1024], bf16, tag="exb2", bufs=6,
                                         name=f"ex{pair}_{qt}_{g}_1")
                        nc.scalar.activation(ex1_t[:], scs[1][:], AF.Exp,
                                             scale=SCALE)
                        ex1 = ex1_t[:]
                    exs = [ex0, ex1]
                    if pending is not None:
                        emit_av(*pending)
                    pending = (u, g, exs)
            emit_av(*pending)
            flush_muls(keep=0)
            while fillers:
                fillers.pop(0)[1]()

    nc.compile()
    return nc


def _host_prep(x, w_qkv, w_proj, b_proj):
    import ml_dtypes
    bf16 = ml_dtypes.bfloat16
    wqkvT = np.ascontiguousarray(w_qkv.T).astype(bf16)   # [C, 3C]
    wpT_full = np.ascontiguousarray(w_proj.T).astype(bf16)  # [C(in), C(out)]
    in_maps = []
    for c in range(NCORES):
        b, g = divmod(c, GROUPS)
        qcols = wqkvT[:, CG * g:CG * (g + 1)]
        kcols = wqkvT[:, C + CG * g:C + CG * (g + 1)]
        vcols = wqkvT[:, 2 * C + CG * g:2 * C + CG * (g + 1)]
        wqk = np.ascontiguousarray(np.concatenate([qcols, kcols], axis=1))
        wv = np.ascontiguousarray(vcols)
        wp = np.ascontiguousarray(wpT_full[CG * g:CG * (g + 1), :])
        xTv = np.ascontiguousarray(x[b].T).astype(bf16)
        in_maps.append({"xT": xTv, "wqkT": wqk, "wvT": wv, "wpT": wp})
    return in_maps


def run(inputs, trace=False, nc=None):
    """Build (or reuse) the program, run on 8 cores, return (y, results)."""
    global _CACHED_NC
    from concourse.bass_utils import run_bass_kernel_spmd
    if nc is None:
        if _CACHED_NC is None:
            _CACHED_NC = _build_nc()
        nc = _CACHED_NC
    in_maps = _host_prep(**inputs)
    res = run_bass_kernel_spmd(nc, in_maps, core_ids=list(range(NCORES)),
                               trace=trace)
    bias = np.asarray(inputs["b_proj"], np.float32)
    out = np.empty((B, N, C), np.float32)
    for b in range(B):
        acc = res.results[b * GROUPS]["yp"].astype(np.float32)
        for g in range(1, GROUPS):
            acc = acc + res.results[b * GROUPS + g]["yp"]
        out[b] = acc + bias
    return out, res


def kernel(x, w_qkv, w_proj, b_proj):
    out, _ = run({"x": np.asarray(x), "w_qkv": np.asarray(w_qkv),
                  "w_proj": np.asarray(w_proj), "b_proj": np.asarray(b_proj)})
    return out
